# revision 13
# baseline (speedup 1.0000x reference)
"""Trainium2 Bass kernel for the SMPL-style kinematic chain problem.

kernel(theta, rest_pose, bone_factor, parents) -> (kp3d, orient, l2ws)

Strategy
--------
Pure data parallel over batch B=131072 across 8 NeuronCores (16384 lanes per
core).  On each core, lanes live on [128 partitions x T free]; every scalar of
the computation is an elementwise plane op over lanes.

Per lane:
  1. Rodrigues via half-angle quaternion, with sin(t/2)/t and 2*cos(t/2)
     evaluated as polynomials in n2 = |theta|^2 (both are entire functions of
     n2) -- no sqrt, no reciprocal, no ACT table-accuracy risk.
  2. Chain compose in depth groups (2-3 joints per instruction via strided
     access patterns).  Third rotation column via cross product (saves 6
     mults/child).  Translations via fused scalar_tensor_tensor with per-joint
     bone-vector scalars.
  3. Results written directly (strided, free at 1x) into an interleaved
     staging tile laid out exactly like the output DRAM, then DMA'd out in two
     joint-sections (joints 0-14: 960B runs, joints 15-23: 576B runs).

Host-side prep is O(NJ) only: bone vectors from rest_pose/bone_factor/parents.
"""

import os
import sys
import time

import numpy as np

for _p in ("/opt/trn_rl_repo",):
    if _p not in sys.path:
        sys.path.insert(0, _p)

B = 131072
NJ = 24
NCORES = 8
BC = B // NCORES  # 16384 lanes per core

# SMPL parent tree: PARENT_IDS[i] is the parent joint of joint i+1.
PARENT_IDS = np.array(
    [0, 0, 0, 1, 2, 3, 4, 5, 6, 7, 8, 9, 9, 9, 12, 13, 14, 16, 17, 18, 19, 20, 21],
    dtype=np.int32,
)

# Depth groups for the chain compose: (child0, n, parent0, parent_step).
GROUPS = [
    (1, 3, 0, 0),
    (4, 3, 1, 1),
    (7, 3, 4, 1),
    (10, 3, 7, 1),
    (13, 2, 9, 0),
    (15, 3, 12, 1),
    (18, 2, 16, 1),
    (20, 2, 18, 1),
    (22, 2, 20, 1),
]

# Rodrigues blocks (contiguous joint ranges; each GROUP's children fall inside
# exactly one block): (j0, nj)
ROD_BLOCKS = [(0, 4), (4, 3), (7, 6), (13, 2), (15, 5), (20, 4)]
NJ_BLK = max(n for _, n in ROD_BLOCKS)  # 6

# Output sections: (first_joint, njoints).  DMA'd separately so the staging
# tile stays small.  Low ends at joint 14 (ready after group (13,2)).
SEC_LOW = (0, 15)
SEC_HIGH = (15, 9)

# cval column layout: [bvx(23) | bvy(23) | bvz(23) | t0(3)]
CV_BVX, CV_BVY, CV_BVZ, CV_T0 = 0, 23, 46, 69
CV_COLS = 72


def _check_groups():
    seen = {}
    for c0, n, p0, ps in GROUPS:
        for i in range(n):
            seen[c0 + i] = p0 + i * ps
    assert sorted(seen) == list(range(1, NJ))
    for c, p in seen.items():
        assert p == int(PARENT_IDS[c - 1]), (c, p, int(PARENT_IDS[c - 1]))
    for c0, n, _, _ in GROUPS:
        assert any(j0 <= c0 and c0 + n <= j0 + nj for j0, nj in ROD_BLOCKS)


_check_groups()


def _fit_poly(f, lo, hi, deg, npts=4001):
    """Weighted-relative least-squares poly fit of f on [lo, hi].
    Returns (coeffs ascending, max relative error)."""
    k = np.arange(npts)
    x = (lo + hi) / 2 + (hi - lo) / 2 * np.cos(np.pi * (k + 0.5) / npts)
    x = np.sort(x)
    y = f(x)
    V = np.stack([x**i for i in range(deg + 1)], axis=1)
    w = 1.0 / np.abs(y)
    c, *_ = np.linalg.lstsq(V * w[:, None], y * w, rcond=None)
    rel = np.abs(V @ c - y) / np.abs(y)
    return c, float(rel.max())


def _poly_coeffs():
    """Polynomials in n2 = |theta|^2 on [0, 6]:
    k(n2)  = sin(sqrt(n2)/2)/sqrt(n2)   (so  q_xyz = theta * k)
    dw(n2) = 2*cos(sqrt(n2)/2)          (so  2*q_w  directly)
    """

    def fk(n2):
        t = np.sqrt(n2)
        return np.sin(t / 2) / t

    def fw(n2):
        t = np.sqrt(n2)
        return 2 * np.cos(t / 2)

    lo, hi = 1e-8, 6.0
    for deg in range(4, 9):
        ck, ek = _fit_poly(fk, lo, hi, deg)
        cw, ew = _fit_poly(fw, lo, hi, deg)
        if max(ek, ew) < 2e-8:
            return ck, cw, deg, max(ek, ew)
    return ck, cw, deg, max(ek, ew)


K_COEF, W_COEF, POLY_DEG, POLY_ERR = _poly_coeffs()


# --------------------------------------------------------------------------
# Program builder
# --------------------------------------------------------------------------

_PROG_CACHE = {}


def build_program(T=64, ntiles=2, repeat=1, use_act=False, use_gpsimd=False):
    """Build the Bass/Tile program for one core: BCt = 128*T*ntiles lanes."""
    key = (T, ntiles, repeat, use_act, use_gpsimd)
    if key in _PROG_CACHE:
        return _PROG_CACHE[key]

    import concourse.bass as bass
    import concourse.tile as tile_mod
    from concourse import bacc, mybir

    f32 = mybir.dt.float32
    Alu = mybir.AluOpType
    Act = mybir.ActivationFunctionType

    BCt = 128 * T * ntiles
    SL = SEC_LOW[1] * 16  # 240
    SH = SEC_HIGH[1] * 16  # 144

    nc = bacc.Bacc("TRN2", target_bir_lowering=False, debug=False)

    theta_in = nc.dram_tensor("theta_in", [BCt, 72], f32, kind="ExternalInput").ap()
    cval_in = nc.dram_tensor("cval", [128, CV_COLS], f32, kind="ExternalInput").ap()
    out_d = nc.dram_tensor("out", [BCt * repeat, 384], f32, kind="ExternalOutput").ap()

    theta_v = theta_in.rearrange("(n p t) c -> n p t c", p=128, t=T)
    out_v = out_d.rearrange("(n p t) s -> n p t s", p=128, t=T)

    deg = POLY_DEG

    from contextlib import ExitStack

    with tile_mod.TileContext(nc) as tc, ExitStack() as ctx:
        constp = ctx.enter_context(tc.tile_pool(name="constp", bufs=1))
        thp = ctx.enter_context(tc.tile_pool(name="thp", bufs=max(2, ntiles)))
        stp = ctx.enter_context(tc.tile_pool(name="stp", bufs=1))
        rodp = ctx.enter_context(tc.tile_pool(name="rodp", bufs=1))
        scrp = ctx.enter_context(tc.tile_pool(name="scrp", bufs=1))

        cv = constp.tile([128, CV_COLS], f32, name="cv")
        nc.sync.dma_start(out=cv[:], in_=cval_in)

        def cvcol(i):
            return cv[:, i : i + 1]

        # Rodrigues scratch planes, [128, T, NJ_BLK] each.
        plane_names = (
            ["sqx", "sqy", "sqz", "nA", "n2"]
            + ["u%d" % i for i in range(2, deg + 1)]
            + ["kp", "kq", "wp", "wq", "qx", "qy", "qz"]
            + ["pxy", "pxz", "pyz", "wx", "wy", "wz", "pxx", "pyy", "pzz"]
            + ["s1", "s2", "s3"]
            + ["r00", "r01", "r02", "r10", "r11", "r12", "r20", "r21", "r22"]
        )
        P = {}
        for pn in plane_names:
            P[pn] = rodp.tile([128, T, NJ_BLK], f32, name="pl_" + pn)

        g1 = scrp.tile([128, T, 3], f32, name="g1")
        g2 = scrp.tile([128, T, 3], f32, name="g2")
        g3 = scrp.tile([128, T, 3], f32, name="g3")
        tsc1 = scrp.tile([128, T], f32, name="tsc1")
        tsc2 = scrp.tile([128, T], f32, name="tsc2")

        ENTRIES = [r * 4 + c for r in range(3) for c in range(3)]

        def emit_rodrigues(th4, j0, nj):
            """theta block [j0, j0+nj) -> local rotation planes P['rXY'][:, :, :nj]."""
            s = lambda pl: P[pl][:, :, 0:nj]
            x = th4[:, :, j0 : j0 + nj, 0]
            y = th4[:, :, j0 : j0 + nj, 1]
            z = th4[:, :, j0 : j0 + nj, 2]
            if use_act:
                nc.scalar.activation(s("sqx"), x, Act.Square)
                nc.scalar.activation(s("sqy"), y, Act.Square)
                nc.scalar.activation(s("sqz"), z, Act.Square)
            else:
                nc.vector.tensor_mul(s("sqx"), x, x)
                nc.vector.tensor_mul(s("sqy"), y, y)
                nc.vector.tensor_mul(s("sqz"), z, z)
            nc.vector.tensor_add(s("nA"), s("sqx"), s("sqy"))
            nc.vector.tensor_add(s("n2"), s("nA"), s("sqz"))
            # powers of n2
            nc.vector.tensor_mul(s("u2"), s("n2"), s("n2"))
            if deg >= 3:
                nc.vector.tensor_mul(s("u3"), s("u2"), s("n2"))
            if deg >= 4:
                nc.vector.tensor_mul(s("u4"), s("u2"), s("u2"))
            if deg >= 5:
                nc.vector.tensor_mul(s("u5"), s("u3"), s("u2"))
            if deg >= 6:
                nc.vector.tensor_mul(s("u6"), s("u3"), s("u3"))
            if deg >= 7:
                nc.vector.tensor_mul(s("u7"), s("u4"), s("u3"))
            if deg >= 8:
                nc.vector.tensor_mul(s("u8"), s("u4"), s("u4"))

            def poly(coefs, a, b):
                # Evaluate sum coefs[i]*n2^i using ping-pong planes a, b.
                nc.vector.tensor_scalar(
                    s(a), s("n2"), float(coefs[1]), float(coefs[0]), Alu.mult, Alu.add
                )
                cur, nxt = a, b
                for i in range(2, len(coefs)):
                    nc.vector.scalar_tensor_tensor(
                        s(nxt), s("u%d" % i), float(coefs[i]), s(cur), Alu.mult, Alu.add
                    )
                    cur, nxt = nxt, cur
                return cur

            kf = poly(K_COEF, "kp", "kq")
            wf = poly(W_COEF, "wp", "wq")
            nc.vector.tensor_mul(s("qx"), x, s(kf))
            nc.vector.tensor_mul(s("qy"), y, s(kf))
            nc.vector.tensor_mul(s("qz"), z, s(kf))
            nc.vector.tensor_mul(s("pxy"), s("qx"), s("qy"))
            nc.vector.tensor_mul(s("pxz"), s("qx"), s("qz"))
            nc.vector.tensor_mul(s("pyz"), s("qy"), s("qz"))
            nc.vector.tensor_mul(s("wx"), s(wf), s("qx"))
            nc.vector.tensor_mul(s("wy"), s(wf), s("qy"))
            nc.vector.tensor_mul(s("wz"), s(wf), s("qz"))
            nc.vector.tensor_mul(s("pxx"), s("qx"), s("qx"))
            nc.vector.tensor_mul(s("pyy"), s("qy"), s("qy"))
            nc.vector.tensor_mul(s("pzz"), s("qz"), s("qz"))
            nc.vector.tensor_add(s("s1"), s("pyy"), s("pzz"))
            nc.vector.tensor_add(s("s2"), s("pxx"), s("pzz"))
            nc.vector.tensor_add(s("s3"), s("pxx"), s("pyy"))
            # diagonals: r = 1 - 2*s
            if use_act:
                nc.scalar.activation(s("r00"), s("s1"), Act.Copy, bias=1.0, scale=-2.0)
                nc.scalar.activation(s("r11"), s("s2"), Act.Copy, bias=1.0, scale=-2.0)
                nc.scalar.activation(s("r22"), s("s3"), Act.Copy, bias=1.0, scale=-2.0)
            else:
                nc.vector.tensor_scalar(s("r00"), s("s1"), -2.0, 1.0, Alu.mult, Alu.add)
                nc.vector.tensor_scalar(s("r11"), s("s2"), -2.0, 1.0, Alu.mult, Alu.add)
                nc.vector.tensor_scalar(s("r22"), s("s3"), -2.0, 1.0, Alu.mult, Alu.add)
            # off-diagonals: wz/wy/wx are already 2*qw*q_
            nc.vector.scalar_tensor_tensor(
                s("r01"), s("pxy"), 2.0, s("wz"), Alu.mult, Alu.subtract
            )
            nc.vector.scalar_tensor_tensor(
                s("r10"), s("pxy"), 2.0, s("wz"), Alu.mult, Alu.add
            )
            nc.vector.scalar_tensor_tensor(
                s("r02"), s("pxz"), 2.0, s("wy"), Alu.mult, Alu.add
            )
            nc.vector.scalar_tensor_tensor(
                s("r20"), s("pxz"), 2.0, s("wy"), Alu.mult, Alu.subtract
            )
            nc.vector.scalar_tensor_tensor(
                s("r12"), s("pyz"), 2.0, s("wx"), Alu.mult, Alu.subtract
            )
            nc.vector.scalar_tensor_tensor(
                s("r21"), s("pyz"), 2.0, s("wx"), Alu.mult, Alu.add
            )

        RNAME = {r * 4 + c: "r%d%d" % (r, c) for r in range(3) for c in range(3)}

        def stage_of(j):
            return 0 if j < SEC_LOW[1] else 1

        def emit_group(c0, n, p0, pstep, blk_j0, stages):
            """Compose children [c0, c0+n) from parents [p0, p0+n*pstep)."""
            stg_out, osec = stages[stage_of(c0)], (SEC_LOW if c0 < 15 else SEC_HIGH)
            stg_par, psec = stages[stage_of(p0)], (SEC_LOW if p0 < 15 else SEC_HIGH)
            lo = c0 - blk_j0

            def par(e):
                if pstep == 1:
                    return stg_par[:, :, p0 - psec[0] : p0 - psec[0] + n, e]
                ap = stg_par[:, :, p0 - psec[0] : p0 - psec[0] + 1, e]
                return ap.broadcast_to((128, T, n))

            def chl(e):
                return P[RNAME[e]][:, :, lo : lo + n]

            def outp(e):
                return stg_out[:, :, c0 - osec[0] : c0 - osec[0] + n, e]

            # columns 0,1 of R_w
            for c in range(2):
                for r in range(3):
                    nc.vector.tensor_mul(g1[:, :, 0:n], par(r * 4 + 0), chl(0 * 4 + c))
                    nc.vector.tensor_mul(g2[:, :, 0:n], par(r * 4 + 1), chl(1 * 4 + c))
                    nc.vector.tensor_add(g3[:, :, 0:n], g1[:, :, 0:n], g2[:, :, 0:n])
                    nc.vector.tensor_mul(g1[:, :, 0:n], par(r * 4 + 2), chl(2 * 4 + c))
                    nc.vector.tensor_add(outp(r * 4 + c), g3[:, :, 0:n], g1[:, :, 0:n])
            # column 2 = col0 x col1 (reads back from stage)
            for r in range(3):
                a1, a2 = (r + 1) % 3, (r + 2) % 3
                nc.vector.tensor_mul(g1[:, :, 0:n], outp(a1 * 4 + 0), outp(a2 * 4 + 1))
                nc.vector.tensor_mul(g2[:, :, 0:n], outp(a2 * 4 + 0), outp(a1 * 4 + 1))
                nc.vector.tensor_sub(outp(r * 4 + 2), g1[:, :, 0:n], g2[:, :, 0:n])
            # column 3: t_w = Rp @ bv + t_p, per child (per-joint bv scalars)
            for i in range(n):
                j = c0 + i
                pj = p0 + i * pstep
                for r in range(3):
                    rp = lambda m: stg_par[:, :, pj - psec[0], r * 4 + m]
                    tp = stg_par[:, :, pj - psec[0], r * 4 + 3]
                    nc.vector.scalar_tensor_tensor(
                        tsc1[:], rp(0), cvcol(CV_BVX + j - 1), tp, Alu.mult, Alu.add
                    )
                    nc.vector.scalar_tensor_tensor(
                        tsc2[:], rp(1), cvcol(CV_BVY + j - 1), tsc1[:], Alu.mult, Alu.add
                    )
                    nc.vector.scalar_tensor_tensor(
                        stg_out[:, :, j - osec[0], r * 4 + 3],
                        rp(2),
                        cvcol(CV_BVZ + j - 1),
                        tsc2[:],
                        Alu.mult,
                        Alu.add,
                    )

        blk_of = {}
        for j0, nj in ROD_BLOCKS:
            for j in range(j0, j0 + nj):
                blk_of[j] = j0

        for ti in range(ntiles):
            th = thp.tile([128, T, 72], f32, name="th")
            nc.sync.dma_start(out=th[:], in_=theta_v[ti])
            for _rep in range(repeat):
                oti = _rep * ntiles + ti  # disjoint DRAM slice per repeat
                th4 = th.rearrange("p t (j c) -> p t j c", c=3)

                stageL = stp.tile([128, T, SL], f32, name="stageL")
                stageH = stp.tile([128, T, SH], f32, name="stageH")
                sL4 = stageL.rearrange("p t (j e) -> p t j e", e=16)
                sH4 = stageH.rearrange("p t (j e) -> p t j e", e=16)
                stages = [sL4, sH4]

                # constant bottom rows [0,0,0,1]
                mse = nc.gpsimd if use_gpsimd else nc.vector
                mse.memset(sL4[:, :, :, 12:15], 0.0)
                mse.memset(sL4[:, :, :, 15], 1.0)
                mse.memset(sH4[:, :, :, 12:15], 0.0)
                mse.memset(sH4[:, :, :, 15], 1.0)

                gi = 0
                for bi, (j0, nj) in enumerate(ROD_BLOCKS):
                    emit_rodrigues(th4, j0, nj)
                    if j0 == 0:
                        # root: l2w[0] = [R_loc[0] | t0]
                        for e in ENTRIES:
                            nc.vector.tensor_copy(
                                sL4[:, :, 0, e], P[RNAME[e]][:, :, 0]
                            )
                        for r in range(3):
                            nc.vector.tensor_scalar(
                                sL4[:, :, 0, r * 4 + 3], P["n2"][:, :, 0],
                                0.0, cvcol(CV_T0 + r), Alu.mult, Alu.add,
                            )
                    # groups whose children live in this block
                    while gi < len(GROUPS) and blk_of[GROUPS[gi][0]] == j0:
                        c0, n, p0, pstep = GROUPS[gi]
                        emit_group(c0, n, p0, pstep, j0, stages)
                        gi += 1
                    if (j0, nj) == (13, 2):
                        nc.sync.dma_start(
                            out=out_v[oti][:, :, 0:SL], in_=stageL[:]
                        )
                assert gi == len(GROUPS)
                nc.sync.dma_start(out=out_v[oti][:, :, SL : SL + SH], in_=stageH[:])

    nc.compile()
    _PROG_CACHE[key] = nc
    return nc


# --------------------------------------------------------------------------
# Host-side driver
# --------------------------------------------------------------------------


def _host_consts(rest_pose, bone_factor, parents):
    rp = np.asarray(rest_pose, dtype=np.float32)
    bf = np.asarray(bone_factor, dtype=np.float32).reshape(NJ - 1)
    par = np.asarray(parents, dtype=np.int64).reshape(NJ - 1)
    bfp = np.sqrt(bf * bf + np.float32(1e-36)).astype(np.float32)
    bv = (rp[1:] - rp[par]) * bfp[:, None]  # (23, 3)
    cval = np.zeros((128, CV_COLS), dtype=np.float32)
    cval[:, CV_BVX : CV_BVX + 23] = bv[:, 0]
    cval[:, CV_BVY : CV_BVY + 23] = bv[:, 1]
    cval[:, CV_BVZ : CV_BVZ + 23] = bv[:, 2]
    cval[:, CV_T0 : CV_T0 + 3] = rp[0]
    return cval


def kernel(theta, rest_pose, bone_factor, parents):
    from concourse.bass_utils import run_bass_kernel_spmd

    theta = np.ascontiguousarray(np.asarray(theta, dtype=np.float32))
    assert theta.shape == (B, NJ, 3)
    cval = _host_consts(rest_pose, bone_factor, parents)

    nc = build_program(T=64, ntiles=2, repeat=1)

    th_flat = theta.reshape(NCORES, BC, 72)
    in_maps = [{"theta_in": th_flat[c], "cval": cval} for c in range(NCORES)]
    res = run_bass_kernel_spmd(nc, in_maps, list(range(NCORES)))
    outs = [res.results[c]["out"] for c in range(NCORES)]
    l2ws = np.concatenate(outs, axis=0).reshape(B, NJ, 4, 4)
    kp3d = l2ws[..., :3, 3]
    orient = l2ws[..., :3, :3]
    return kp3d, orient, l2ws


# --------------------------------------------------------------------------
# Pure-numpy emulation of the device math (for simulator tests)
# --------------------------------------------------------------------------


def numpy_equivalent(theta, rest_pose, bone_factor, parents):
    """Same math as the device kernel, in float64-ish numpy (for debugging)."""
    th = np.asarray(theta, np.float32).astype(np.float64)
    Bd = th.shape[0]
    n2 = (th * th).sum(-1)  # (B, NJ)
    k = np.zeros_like(n2)
    w = np.zeros_like(n2)
    for i, c in enumerate(K_COEF):
        k += c * n2**i
    for i, c in enumerate(W_COEF):
        w += c * n2**i
    q = th * k[..., None]  # (B,NJ,3)
    qx, qy, qz = q[..., 0], q[..., 1], q[..., 2]
    wx, wy, wz = w * qx, w * qy, w * qz
    R = np.zeros((Bd, NJ, 3, 3))
    R[..., 0, 0] = 1 - 2 * (qy * qy + qz * qz)
    R[..., 1, 1] = 1 - 2 * (qx * qx + qz * qz)
    R[..., 2, 2] = 1 - 2 * (qx * qx + qy * qy)
    R[..., 0, 1] = 2 * qx * qy - wz
    R[..., 1, 0] = 2 * qx * qy + wz
    R[..., 0, 2] = 2 * qx * qz + wy
    R[..., 2, 0] = 2 * qx * qz - wy
    R[..., 1, 2] = 2 * qy * qz - wx
    R[..., 2, 1] = 2 * qy * qz + wx

    rp = np.asarray(rest_pose, np.float64)
    bf = np.asarray(bone_factor, np.float64).reshape(NJ - 1)
    par = np.asarray(parents, np.int64).reshape(NJ - 1)
    bv = (rp[1:] - rp[par]) * np.sqrt(bf * bf + 1e-36)[:, None]

    l2ws = np.zeros((Bd, NJ, 4, 4))
    l2ws[:, :, 3, 3] = 1.0
    l2ws[:, 0, :3, :3] = R[:, 0]
    l2ws[:, 0, :3, 3] = rp[0]
    for c0, n, p0, pstep in GROUPS:
        for i in range(n):
            j, pj = c0 + i, p0 + i * pstep
            Rw = l2ws[:, pj, :3, :3] @ R[:, j]
            Rw[:, :, 2] = np.cross(Rw[:, :, 0], Rw[:, :, 1])
            l2ws[:, j, :3, :3] = Rw
            l2ws[:, j, :3, 3] = (
                np.einsum("bij,j->bi", l2ws[:, pj, :3, :3], bv[j - 1])
                + l2ws[:, pj, :3, 3]
            )
    kp3d = l2ws[..., :3, 3]
    orient = l2ws[..., :3, :3]
    return kp3d.astype(np.float32), orient.astype(np.float32), l2ws.astype(np.float32)


if __name__ == "__main__":
    print(f"poly degree {POLY_DEG}, fit rel err {POLY_ERR:.2e}")


# revision 15
# speedup vs baseline: 84.2920x; 84.2920x over previous
"""Trainium2 Bass kernel for the SMPL-style kinematic chain problem.

kernel(theta, rest_pose, bone_factor, parents) -> (kp3d, orient, l2ws)

Strategy
--------
Pure data parallel over batch B=131072 across 8 NeuronCores (16384 lanes per
core).  On each core, lanes live on [128 partitions x T free]; every scalar of
the computation is an elementwise plane op over lanes.

Per lane:
  1. Rodrigues via half-angle quaternion, with sin(t/2)/t and 2*cos(t/2)
     evaluated as polynomials in n2 = |theta|^2 (both are entire functions of
     n2) -- no sqrt, no reciprocal, no ACT table-accuracy risk.
  2. Chain compose in depth groups (2-3 joints per instruction via strided
     access patterns).  Third rotation column via cross product (saves 6
     mults/child).  Translations via fused scalar_tensor_tensor with per-joint
     bone-vector scalars.
  3. Results written directly (strided, free at 1x) into an interleaved
     staging tile laid out exactly like the output DRAM, then DMA'd out in two
     joint-sections (joints 0-14: 960B runs, joints 15-23: 576B runs).

Host-side prep is O(NJ) only: bone vectors from rest_pose/bone_factor/parents.
"""

import os
import sys
import time

import numpy as np

for _p in ("/opt/trn_rl_repo",):
    if _p not in sys.path:
        sys.path.insert(0, _p)

B = 131072
NJ = 24
NCORES = 8
BC = B // NCORES  # 16384 lanes per core

# SMPL parent tree: PARENT_IDS[i] is the parent joint of joint i+1.
PARENT_IDS = np.array(
    [0, 0, 0, 1, 2, 3, 4, 5, 6, 7, 8, 9, 9, 9, 12, 13, 14, 16, 17, 18, 19, 20, 21],
    dtype=np.int32,
)

# Depth groups for the chain compose: (child0, n, parent0, parent_step).
GROUPS = [
    (1, 3, 0, 0),
    (4, 3, 1, 1),
    (7, 3, 4, 1),
    (10, 3, 7, 1),
    (13, 2, 9, 0),
    (15, 3, 12, 1),
    (18, 2, 16, 1),
    (20, 2, 18, 1),
    (22, 2, 20, 1),
]

# Rodrigues blocks (contiguous joint ranges; each GROUP's children fall inside
# exactly one block): (j0, nj)
ROD_BLOCKS = [(0, 4), (4, 3), (7, 6), (13, 2), (15, 5), (20, 4)]
NJ_BLK = max(n for _, n in ROD_BLOCKS)  # 6

# Output sections: (first_joint, njoints).  DMA'd separately so the staging
# tile stays small.  Low ends at joint 14 (ready after group (13,2)).
SEC_LOW = (0, 15)
SEC_HIGH = (15, 9)

# cval column layout: [bvx(23) | bvy(23) | bvz(23) | t0(3)]
CV_BVX, CV_BVY, CV_BVZ, CV_T0 = 0, 23, 46, 69
CV_COLS = 72


def _check_groups():
    seen = {}
    for c0, n, p0, ps in GROUPS:
        for i in range(n):
            seen[c0 + i] = p0 + i * ps
    assert sorted(seen) == list(range(1, NJ))
    for c, p in seen.items():
        assert p == int(PARENT_IDS[c - 1]), (c, p, int(PARENT_IDS[c - 1]))
    for c0, n, _, _ in GROUPS:
        assert any(j0 <= c0 and c0 + n <= j0 + nj for j0, nj in ROD_BLOCKS)


_check_groups()


def _fit_poly(f, lo, hi, deg, npts=4001):
    """Weighted-relative least-squares poly fit of f on [lo, hi].
    Returns (coeffs ascending, max relative error)."""
    k = np.arange(npts)
    x = (lo + hi) / 2 + (hi - lo) / 2 * np.cos(np.pi * (k + 0.5) / npts)
    x = np.sort(x)
    y = f(x)
    V = np.stack([x**i for i in range(deg + 1)], axis=1)
    w = 1.0 / np.abs(y)
    c, *_ = np.linalg.lstsq(V * w[:, None], y * w, rcond=None)
    rel = np.abs(V @ c - y) / np.abs(y)
    return c, float(rel.max())


def _poly_coeffs():
    """Polynomials in n2 = |theta|^2 on [0, 6]:
    k(n2)  = sin(sqrt(n2)/2)/sqrt(n2)   (so  q_xyz = theta * k)
    dw(n2) = 2*cos(sqrt(n2)/2)          (so  2*q_w  directly)
    """

    def fk(n2):
        t = np.sqrt(n2)
        return np.sin(t / 2) / t

    def fw(n2):
        t = np.sqrt(n2)
        return 2 * np.cos(t / 2)

    lo, hi = 1e-8, 6.0
    for deg in range(4, 9):
        ck, ek = _fit_poly(fk, lo, hi, deg)
        cw, ew = _fit_poly(fw, lo, hi, deg)
        if max(ek, ew) < 2e-8:
            return ck, cw, deg, max(ek, ew)
    return ck, cw, deg, max(ek, ew)


K_COEF, W_COEF, POLY_DEG, POLY_ERR = _poly_coeffs()


# --------------------------------------------------------------------------
# Program builder
# --------------------------------------------------------------------------

_PROG_CACHE = {}


def build_program(T=64, ntiles=2, repeat=1, use_act=False, use_gpsimd=False):
    """Build the Bass/Tile program for one core: BCt = 128*T*ntiles lanes."""
    key = (T, ntiles, repeat, use_act, use_gpsimd)
    if key in _PROG_CACHE:
        return _PROG_CACHE[key]

    import concourse.bass as bass
    import concourse.tile as tile_mod
    from concourse import bacc, mybir

    f32 = mybir.dt.float32
    Alu = mybir.AluOpType
    Act = mybir.ActivationFunctionType

    BCt = 128 * T * ntiles
    SL = SEC_LOW[1] * 16  # 240
    SH = SEC_HIGH[1] * 16  # 144

    nc = bacc.Bacc("TRN2", target_bir_lowering=False, debug=False)

    theta_in = nc.dram_tensor("theta_in", [BCt, 72], f32, kind="ExternalInput").ap()
    cval_in = nc.dram_tensor("cval", [128, CV_COLS], f32, kind="ExternalInput").ap()
    out_d = nc.dram_tensor("out", [BCt, 384], f32, kind="ExternalOutput").ap()

    theta_v = theta_in.rearrange("(n p t) c -> n p t c", p=128, t=T)
    out_v = out_d.rearrange("(n p t) s -> n p t s", p=128, t=T)

    deg = POLY_DEG

    from contextlib import ExitStack

    with tile_mod.TileContext(nc) as tc, ExitStack() as ctx:
        constp = ctx.enter_context(tc.tile_pool(name="constp", bufs=1))
        thp = ctx.enter_context(tc.tile_pool(name="thp", bufs=max(2, ntiles)))
        stp = ctx.enter_context(tc.tile_pool(name="stp", bufs=1))
        rodp = ctx.enter_context(tc.tile_pool(name="rodp", bufs=1))
        scrp = ctx.enter_context(tc.tile_pool(name="scrp", bufs=1))

        cv = constp.tile([128, CV_COLS], f32, name="cv")
        nc.sync.dma_start(out=cv[:], in_=cval_in)

        def cvcol(i):
            return cv[:, i : i + 1]

        # Rodrigues scratch planes, [128, T, NJ_BLK] each.
        plane_names = (
            ["sqx", "sqy", "sqz", "nA", "n2"]
            + ["u%d" % i for i in range(2, deg + 1)]
            + ["kp", "kq", "wp", "wq", "qx", "qy", "qz"]
            + ["pxy", "pxz", "pyz", "wx", "wy", "wz", "pxx", "pyy", "pzz"]
            + ["s1", "s2", "s3"]
            + ["r00", "r01", "r02", "r10", "r11", "r12", "r20", "r21", "r22"]
        )
        P = {}
        for pn in plane_names:
            P[pn] = rodp.tile([128, T, NJ_BLK], f32, name="pl_" + pn)

        g1 = scrp.tile([128, T, 3], f32, name="g1")
        g2 = scrp.tile([128, T, 3], f32, name="g2")
        g3 = scrp.tile([128, T, 3], f32, name="g3")
        tsc1 = scrp.tile([128, T], f32, name="tsc1")
        tsc2 = scrp.tile([128, T], f32, name="tsc2")

        ENTRIES = [r * 4 + c for r in range(3) for c in range(3)]

        def emit_rodrigues(th4, j0, nj):
            """theta block [j0, j0+nj) -> local rotation planes P['rXY'][:, :, :nj]."""
            s = lambda pl: P[pl][:, :, 0:nj]
            x = th4[:, :, j0 : j0 + nj, 0]
            y = th4[:, :, j0 : j0 + nj, 1]
            z = th4[:, :, j0 : j0 + nj, 2]
            if use_act:
                nc.scalar.activation(s("sqx"), x, Act.Square)
                nc.scalar.activation(s("sqy"), y, Act.Square)
                nc.scalar.activation(s("sqz"), z, Act.Square)
            else:
                nc.vector.tensor_mul(s("sqx"), x, x)
                nc.vector.tensor_mul(s("sqy"), y, y)
                nc.vector.tensor_mul(s("sqz"), z, z)
            nc.vector.tensor_add(s("nA"), s("sqx"), s("sqy"))
            nc.vector.tensor_add(s("n2"), s("nA"), s("sqz"))
            # powers of n2
            nc.vector.tensor_mul(s("u2"), s("n2"), s("n2"))
            if deg >= 3:
                nc.vector.tensor_mul(s("u3"), s("u2"), s("n2"))
            if deg >= 4:
                nc.vector.tensor_mul(s("u4"), s("u2"), s("u2"))
            if deg >= 5:
                nc.vector.tensor_mul(s("u5"), s("u3"), s("u2"))
            if deg >= 6:
                nc.vector.tensor_mul(s("u6"), s("u3"), s("u3"))
            if deg >= 7:
                nc.vector.tensor_mul(s("u7"), s("u4"), s("u3"))
            if deg >= 8:
                nc.vector.tensor_mul(s("u8"), s("u4"), s("u4"))

            def poly(coefs, a, b):
                # Evaluate sum coefs[i]*n2^i using ping-pong planes a, b.
                nc.vector.tensor_scalar(
                    s(a), s("n2"), float(coefs[1]), float(coefs[0]), Alu.mult, Alu.add
                )
                cur, nxt = a, b
                for i in range(2, len(coefs)):
                    nc.vector.scalar_tensor_tensor(
                        s(nxt), s("u%d" % i), float(coefs[i]), s(cur), Alu.mult, Alu.add
                    )
                    cur, nxt = nxt, cur
                return cur

            kf = poly(K_COEF, "kp", "kq")
            wf = poly(W_COEF, "wp", "wq")
            nc.vector.tensor_mul(s("qx"), x, s(kf))
            nc.vector.tensor_mul(s("qy"), y, s(kf))
            nc.vector.tensor_mul(s("qz"), z, s(kf))
            nc.vector.tensor_mul(s("pxy"), s("qx"), s("qy"))
            nc.vector.tensor_mul(s("pxz"), s("qx"), s("qz"))
            nc.vector.tensor_mul(s("pyz"), s("qy"), s("qz"))
            nc.vector.tensor_mul(s("wx"), s(wf), s("qx"))
            nc.vector.tensor_mul(s("wy"), s(wf), s("qy"))
            nc.vector.tensor_mul(s("wz"), s(wf), s("qz"))
            nc.vector.tensor_mul(s("pxx"), s("qx"), s("qx"))
            nc.vector.tensor_mul(s("pyy"), s("qy"), s("qy"))
            nc.vector.tensor_mul(s("pzz"), s("qz"), s("qz"))
            nc.vector.tensor_add(s("s1"), s("pyy"), s("pzz"))
            nc.vector.tensor_add(s("s2"), s("pxx"), s("pzz"))
            nc.vector.tensor_add(s("s3"), s("pxx"), s("pyy"))
            # diagonals: r = 1 - 2*s
            if use_act:
                nc.scalar.activation(s("r00"), s("s1"), Act.Copy, bias=1.0, scale=-2.0)
                nc.scalar.activation(s("r11"), s("s2"), Act.Copy, bias=1.0, scale=-2.0)
                nc.scalar.activation(s("r22"), s("s3"), Act.Copy, bias=1.0, scale=-2.0)
            else:
                nc.vector.tensor_scalar(s("r00"), s("s1"), -2.0, 1.0, Alu.mult, Alu.add)
                nc.vector.tensor_scalar(s("r11"), s("s2"), -2.0, 1.0, Alu.mult, Alu.add)
                nc.vector.tensor_scalar(s("r22"), s("s3"), -2.0, 1.0, Alu.mult, Alu.add)
            # off-diagonals: wz/wy/wx are already 2*qw*q_
            nc.vector.scalar_tensor_tensor(
                s("r01"), s("pxy"), 2.0, s("wz"), Alu.mult, Alu.subtract
            )
            nc.vector.scalar_tensor_tensor(
                s("r10"), s("pxy"), 2.0, s("wz"), Alu.mult, Alu.add
            )
            nc.vector.scalar_tensor_tensor(
                s("r02"), s("pxz"), 2.0, s("wy"), Alu.mult, Alu.add
            )
            nc.vector.scalar_tensor_tensor(
                s("r20"), s("pxz"), 2.0, s("wy"), Alu.mult, Alu.subtract
            )
            nc.vector.scalar_tensor_tensor(
                s("r12"), s("pyz"), 2.0, s("wx"), Alu.mult, Alu.subtract
            )
            nc.vector.scalar_tensor_tensor(
                s("r21"), s("pyz"), 2.0, s("wx"), Alu.mult, Alu.add
            )

        RNAME = {r * 4 + c: "r%d%d" % (r, c) for r in range(3) for c in range(3)}

        def stage_of(j):
            return 0 if j < SEC_LOW[1] else 1

        def emit_group(c0, n, p0, pstep, blk_j0, stages):
            """Compose children [c0, c0+n) from parents [p0, p0+n*pstep)."""
            stg_out, osec = stages[stage_of(c0)], (SEC_LOW if c0 < 15 else SEC_HIGH)
            stg_par, psec = stages[stage_of(p0)], (SEC_LOW if p0 < 15 else SEC_HIGH)
            lo = c0 - blk_j0

            def par(e):
                if pstep == 1:
                    return stg_par[:, :, p0 - psec[0] : p0 - psec[0] + n, e]
                ap = stg_par[:, :, p0 - psec[0] : p0 - psec[0] + 1, e]
                return ap.broadcast_to((128, T, n))

            def chl(e):
                return P[RNAME[e]][:, :, lo : lo + n]

            def outp(e):
                return stg_out[:, :, c0 - osec[0] : c0 - osec[0] + n, e]

            # columns 0,1 of R_w
            for c in range(2):
                for r in range(3):
                    nc.vector.tensor_mul(g1[:, :, 0:n], par(r * 4 + 0), chl(0 * 4 + c))
                    nc.vector.tensor_mul(g2[:, :, 0:n], par(r * 4 + 1), chl(1 * 4 + c))
                    nc.vector.tensor_add(g3[:, :, 0:n], g1[:, :, 0:n], g2[:, :, 0:n])
                    nc.vector.tensor_mul(g1[:, :, 0:n], par(r * 4 + 2), chl(2 * 4 + c))
                    nc.vector.tensor_add(outp(r * 4 + c), g3[:, :, 0:n], g1[:, :, 0:n])
            # column 2 = col0 x col1 (reads back from stage)
            for r in range(3):
                a1, a2 = (r + 1) % 3, (r + 2) % 3
                nc.vector.tensor_mul(g1[:, :, 0:n], outp(a1 * 4 + 0), outp(a2 * 4 + 1))
                nc.vector.tensor_mul(g2[:, :, 0:n], outp(a2 * 4 + 0), outp(a1 * 4 + 1))
                nc.vector.tensor_sub(outp(r * 4 + 2), g1[:, :, 0:n], g2[:, :, 0:n])
            # column 3: t_w = Rp @ bv + t_p, per child (per-joint bv scalars)
            for i in range(n):
                j = c0 + i
                pj = p0 + i * pstep
                for r in range(3):
                    rp = lambda m: stg_par[:, :, pj - psec[0], r * 4 + m]
                    tp = stg_par[:, :, pj - psec[0], r * 4 + 3]
                    nc.vector.scalar_tensor_tensor(
                        tsc1[:], rp(0), cvcol(CV_BVX + j - 1), tp, Alu.mult, Alu.add
                    )
                    nc.vector.scalar_tensor_tensor(
                        tsc2[:], rp(1), cvcol(CV_BVY + j - 1), tsc1[:], Alu.mult, Alu.add
                    )
                    nc.vector.scalar_tensor_tensor(
                        stg_out[:, :, j - osec[0], r * 4 + 3],
                        rp(2),
                        cvcol(CV_BVZ + j - 1),
                        tsc2[:],
                        Alu.mult,
                        Alu.add,
                    )

        blk_of = {}
        for j0, nj in ROD_BLOCKS:
            for j in range(j0, j0 + nj):
                blk_of[j] = j0

        from contextlib import nullcontext

        # repeat>1: timing mode — run the whole pass `repeat` times via a
        # dynamic loop (same inputs/outputs each iteration).
        loop_cm = tc.For_i(0, repeat, 1) if repeat > 1 else nullcontext()
        with loop_cm:
            for ti in range(ntiles):
                th = thp.tile([128, T, 72], f32, name="th")
                nc.sync.dma_start(out=th[:], in_=theta_v[ti])
                oti = ti
                th4 = th.rearrange("p t (j c) -> p t j c", c=3)

                stageL = stp.tile([128, T, SL], f32, name="stageL")
                stageH = stp.tile([128, T, SH], f32, name="stageH")
                sL4 = stageL.rearrange("p t (j e) -> p t j e", e=16)
                sH4 = stageH.rearrange("p t (j e) -> p t j e", e=16)
                stages = [sL4, sH4]

                # constant bottom rows [0,0,0,1]
                mse = nc.gpsimd if use_gpsimd else nc.vector
                mse.memset(sL4[:, :, :, 12:15], 0.0)
                mse.memset(sL4[:, :, :, 15], 1.0)
                mse.memset(sH4[:, :, :, 12:15], 0.0)
                mse.memset(sH4[:, :, :, 15], 1.0)

                gi = 0
                for bi, (j0, nj) in enumerate(ROD_BLOCKS):
                    emit_rodrigues(th4, j0, nj)
                    if j0 == 0:
                        # root: l2w[0] = [R_loc[0] | t0]
                        for e in ENTRIES:
                            nc.vector.tensor_copy(
                                sL4[:, :, 0, e], P[RNAME[e]][:, :, 0]
                            )
                        for r in range(3):
                            nc.vector.tensor_scalar(
                                sL4[:, :, 0, r * 4 + 3], P["n2"][:, :, 0],
                                0.0, cvcol(CV_T0 + r), Alu.mult, Alu.add,
                            )
                    # groups whose children live in this block
                    while gi < len(GROUPS) and blk_of[GROUPS[gi][0]] == j0:
                        c0, n, p0, pstep = GROUPS[gi]
                        emit_group(c0, n, p0, pstep, j0, stages)
                        gi += 1
                    if (j0, nj) == (13, 2):
                        nc.sync.dma_start(
                            out=out_v[oti][:, :, 0:SL], in_=stageL[:]
                        )
                assert gi == len(GROUPS)
                nc.sync.dma_start(out=out_v[oti][:, :, SL : SL + SH], in_=stageH[:])

    nc.compile()
    _PROG_CACHE[key] = nc
    return nc


# --------------------------------------------------------------------------
# Host-side driver
# --------------------------------------------------------------------------


def _host_consts(rest_pose, bone_factor, parents):
    rp = np.asarray(rest_pose, dtype=np.float32)
    bf = np.asarray(bone_factor, dtype=np.float32).reshape(NJ - 1)
    par = np.asarray(parents, dtype=np.int64).reshape(NJ - 1)
    bfp = np.sqrt(bf * bf + np.float32(1e-36)).astype(np.float32)
    bv = (rp[1:] - rp[par]) * bfp[:, None]  # (23, 3)
    cval = np.zeros((128, CV_COLS), dtype=np.float32)
    cval[:, CV_BVX : CV_BVX + 23] = bv[:, 0]
    cval[:, CV_BVY : CV_BVY + 23] = bv[:, 1]
    cval[:, CV_BVZ : CV_BVZ + 23] = bv[:, 2]
    cval[:, CV_T0 : CV_T0 + 3] = rp[0]
    return cval


def kernel(theta, rest_pose, bone_factor, parents):
    from concourse.bass_utils import run_bass_kernel_spmd

    theta = np.ascontiguousarray(np.asarray(theta, dtype=np.float32))
    assert theta.shape == (B, NJ, 3)
    cval = _host_consts(rest_pose, bone_factor, parents)

    nc = build_program(T=64, ntiles=2, repeat=1)

    th_flat = theta.reshape(NCORES, BC, 72)
    in_maps = [{"theta_in": th_flat[c], "cval": cval} for c in range(NCORES)]
    res = run_bass_kernel_spmd(nc, in_maps, list(range(NCORES)))
    outs = [res.results[c]["out"] for c in range(NCORES)]
    l2ws = np.concatenate(outs, axis=0).reshape(B, NJ, 4, 4)
    kp3d = l2ws[..., :3, 3]
    orient = l2ws[..., :3, :3]
    return kp3d, orient, l2ws


# --------------------------------------------------------------------------
# Pure-numpy emulation of the device math (for simulator tests)
# --------------------------------------------------------------------------


def numpy_equivalent(theta, rest_pose, bone_factor, parents):
    """Same math as the device kernel, in float64-ish numpy (for debugging)."""
    th = np.asarray(theta, np.float32).astype(np.float64)
    Bd = th.shape[0]
    n2 = (th * th).sum(-1)  # (B, NJ)
    k = np.zeros_like(n2)
    w = np.zeros_like(n2)
    for i, c in enumerate(K_COEF):
        k += c * n2**i
    for i, c in enumerate(W_COEF):
        w += c * n2**i
    q = th * k[..., None]  # (B,NJ,3)
    qx, qy, qz = q[..., 0], q[..., 1], q[..., 2]
    wx, wy, wz = w * qx, w * qy, w * qz
    R = np.zeros((Bd, NJ, 3, 3))
    R[..., 0, 0] = 1 - 2 * (qy * qy + qz * qz)
    R[..., 1, 1] = 1 - 2 * (qx * qx + qz * qz)
    R[..., 2, 2] = 1 - 2 * (qx * qx + qy * qy)
    R[..., 0, 1] = 2 * qx * qy - wz
    R[..., 1, 0] = 2 * qx * qy + wz
    R[..., 0, 2] = 2 * qx * qz + wy
    R[..., 2, 0] = 2 * qx * qz - wy
    R[..., 1, 2] = 2 * qy * qz - wx
    R[..., 2, 1] = 2 * qy * qz + wx

    rp = np.asarray(rest_pose, np.float64)
    bf = np.asarray(bone_factor, np.float64).reshape(NJ - 1)
    par = np.asarray(parents, np.int64).reshape(NJ - 1)
    bv = (rp[1:] - rp[par]) * np.sqrt(bf * bf + 1e-36)[:, None]

    l2ws = np.zeros((Bd, NJ, 4, 4))
    l2ws[:, :, 3, 3] = 1.0
    l2ws[:, 0, :3, :3] = R[:, 0]
    l2ws[:, 0, :3, 3] = rp[0]
    for c0, n, p0, pstep in GROUPS:
        for i in range(n):
            j, pj = c0 + i, p0 + i * pstep
            Rw = l2ws[:, pj, :3, :3] @ R[:, j]
            Rw[:, :, 2] = np.cross(Rw[:, :, 0], Rw[:, :, 1])
            l2ws[:, j, :3, :3] = Rw
            l2ws[:, j, :3, 3] = (
                np.einsum("bij,j->bi", l2ws[:, pj, :3, :3], bv[j - 1])
                + l2ws[:, pj, :3, 3]
            )
    kp3d = l2ws[..., :3, 3]
    orient = l2ws[..., :3, :3]
    return kp3d.astype(np.float32), orient.astype(np.float32), l2ws.astype(np.float32)


if __name__ == "__main__":
    print(f"poly degree {POLY_DEG}, fit rel err {POLY_ERR:.2e}")


# revision 21
# speedup vs baseline: 106.9355x; 1.2686x over previous
"""Trainium2 Bass kernel for the SMPL-style kinematic chain problem.

kernel(theta, rest_pose, bone_factor, parents) -> (kp3d, orient, l2ws)

Strategy
--------
Pure data parallel over batch B=131072 across 8 NeuronCores (16384 lanes per
core).  On each core, lanes live on [128 partitions x T free]; every scalar of
the computation is an elementwise plane op over lanes.

Per lane:
  1. Rodrigues via half-angle quaternion, with sin(t/2)/t and 2*cos(t/2)
     evaluated as polynomials in n2 = |theta|^2 (both are entire functions of
     n2) -- no sqrt, no reciprocal, no ACT table-accuracy risk.
  2. Chain compose in depth groups (2-3 joints per instruction via strided
     access patterns).  Third rotation column via cross product (saves 6
     mults/child).  Translations via fused scalar_tensor_tensor with per-joint
     bone-vector scalars.
  3. Results written directly (strided, free at 1x) into an interleaved
     staging tile laid out exactly like the output DRAM, then DMA'd out in two
     joint-sections (joints 0-14: 960B runs, joints 15-23: 576B runs).

Host-side prep is O(NJ) only: bone vectors from rest_pose/bone_factor/parents.
"""

import os
import sys
import time

import numpy as np

for _p in ("/opt/trn_rl_repo",):
    if _p not in sys.path:
        sys.path.insert(0, _p)

B = 131072
NJ = 24
NCORES = 8
BC = B // NCORES  # 16384 lanes per core

# SMPL parent tree: PARENT_IDS[i] is the parent joint of joint i+1.
PARENT_IDS = np.array(
    [0, 0, 0, 1, 2, 3, 4, 5, 6, 7, 8, 9, 9, 9, 12, 13, 14, 16, 17, 18, 19, 20, 21],
    dtype=np.int32,
)

# Depth groups for the chain compose: (child0, n, parent0, parent_step).
GROUPS = [
    (1, 3, 0, 0),
    (4, 3, 1, 1),
    (7, 3, 4, 1),
    (10, 3, 7, 1),
    (13, 2, 9, 0),
    (15, 3, 12, 1),
    (18, 2, 16, 1),
    (20, 2, 18, 1),
    (22, 2, 20, 1),
]

# Rodrigues blocks (contiguous joint ranges; each GROUP's children fall inside
# exactly one block): (j0, nj)
ROD_BLOCKS = [(0, 4), (4, 3), (7, 6), (13, 2), (15, 5), (20, 4)]
NJ_BLK = max(n for _, n in ROD_BLOCKS)  # 6

# Output sections: (first_joint, njoints).  DMA'd separately so the staging
# tile stays small.  Low ends at joint 14 (ready after group (13,2)).
SEC_LOW = (0, 15)
SEC_HIGH = (15, 9)

# cval column layout: [bvx(23) | bvy(23) | bvz(23) | t0(3)]
CV_BVX, CV_BVY, CV_BVZ, CV_T0 = 0, 23, 46, 69
CV_COLS = 72


def _check_groups():
    seen = {}
    for c0, n, p0, ps in GROUPS:
        for i in range(n):
            seen[c0 + i] = p0 + i * ps
    assert sorted(seen) == list(range(1, NJ))
    for c, p in seen.items():
        assert p == int(PARENT_IDS[c - 1]), (c, p, int(PARENT_IDS[c - 1]))
    for c0, n, _, _ in GROUPS:
        assert any(j0 <= c0 and c0 + n <= j0 + nj for j0, nj in ROD_BLOCKS)


_check_groups()


def _fit_poly(f, lo, hi, deg, npts=4001):
    """Weighted-relative least-squares poly fit of f on [lo, hi].
    Returns (coeffs ascending, max relative error)."""
    k = np.arange(npts)
    x = (lo + hi) / 2 + (hi - lo) / 2 * np.cos(np.pi * (k + 0.5) / npts)
    x = np.sort(x)
    y = f(x)
    V = np.stack([x**i for i in range(deg + 1)], axis=1)
    w = 1.0 / np.abs(y)
    c, *_ = np.linalg.lstsq(V * w[:, None], y * w, rcond=None)
    rel = np.abs(V @ c - y) / np.abs(y)
    return c, float(rel.max())


def _poly_coeffs():
    """Polynomials in n2 = |theta|^2 on [0, 6]:
    k(n2)  = sin(sqrt(n2)/2)/sqrt(n2)   (so  q_xyz = theta * k)
    dw(n2) = 2*cos(sqrt(n2)/2)          (so  2*q_w  directly)
    """

    def fk(n2):
        t = np.sqrt(n2)
        return np.sin(t / 2) / t

    def fw(n2):
        t = np.sqrt(n2)
        return 2 * np.cos(t / 2)

    lo, hi = 1e-8, 6.0
    for deg in range(4, 9):
        ck, ek = _fit_poly(fk, lo, hi, deg)
        cw, ew = _fit_poly(fw, lo, hi, deg)
        if max(ek, ew) < 2e-8:
            return ck, cw, deg, max(ek, ew)
    return ck, cw, deg, max(ek, ew)


K_COEF, W_COEF, POLY_DEG, POLY_ERR = _poly_coeffs()


# --------------------------------------------------------------------------
# Program builder
# --------------------------------------------------------------------------

_PROG_CACHE = {}


def build_program(T=64, ntiles=2, repeat=1, use_act=True, use_gpsimd=False):
    """Build the Bass/Tile program for one core: BCt = 128*T*ntiles lanes."""
    key = (T, ntiles, repeat, use_act, use_gpsimd)
    if key in _PROG_CACHE:
        return _PROG_CACHE[key]

    import concourse.bass as bass
    import concourse.tile as tile_mod
    from concourse import bacc, mybir

    f32 = mybir.dt.float32
    Alu = mybir.AluOpType
    Act = mybir.ActivationFunctionType

    BCt = 128 * T * ntiles
    SL = SEC_LOW[1] * 16  # 240
    SH = SEC_HIGH[1] * 16  # 144

    nc = bacc.Bacc("TRN2", target_bir_lowering=False, debug=False)

    theta_in = nc.dram_tensor("theta_in", [BCt, 72], f32, kind="ExternalInput").ap()
    cval_in = nc.dram_tensor("cval", [128, CV_COLS], f32, kind="ExternalInput").ap()
    out_d = nc.dram_tensor("out", [BCt, 384], f32, kind="ExternalOutput").ap()

    theta_v = theta_in.rearrange("(n p t) c -> n p t c", p=128, t=T)
    out_v = out_d.rearrange("(n p t) s -> n p t s", p=128, t=T)

    deg = POLY_DEG

    from contextlib import ExitStack

    with tile_mod.TileContext(nc) as tc, ExitStack() as ctx:
        constp = ctx.enter_context(tc.tile_pool(name="constp", bufs=1))
        thp = ctx.enter_context(tc.tile_pool(name="thp", bufs=max(2, ntiles)))
        stp = ctx.enter_context(tc.tile_pool(name="stp", bufs=1))
        rodp = ctx.enter_context(tc.tile_pool(name="rodp", bufs=1))
        scrp = ctx.enter_context(tc.tile_pool(name="scrp", bufs=1))

        cv = constp.tile([128, CV_COLS], f32, name="cv")
        nc.sync.dma_start(out=cv[:], in_=cval_in)

        def cvcol(i):
            return cv[:, i : i + 1]

        # Rodrigues scratch planes, [128, T, NJ_BLK] each.
        plane_names = (
            ["sqx", "sqy", "sqz", "nA", "n2"]
            + ["u%d" % i for i in range(2, deg + 1)]
            + ["kp", "kq", "wp", "wq", "qx", "qy", "qz"]
            + ["pxy", "pxz", "pyz", "wx", "wy", "wz", "pxx", "pyy", "pzz"]
            + ["s1", "s2", "s3"]
        )
        P = {}
        for pn in plane_names:
            P[pn] = rodp.tile([128, T, NJ_BLK], f32, name="pl_" + pn)
        # Local rotations, one tile: entry (r,c) at index r*3+c (row-major).
        Rt = rodp.tile([128, T, NJ_BLK, 9], f32, name="Rt")

        g1 = scrp.tile([128, T, 6], f32, name="g1")
        g2 = scrp.tile([128, T, 6], f32, name="g2")
        g3 = scrp.tile([128, T, 6], f32, name="g3")
        g1v = g1.rearrange("p t (r c) -> p t r c", c=2)
        g2v = g2.rearrange("p t (r c) -> p t r c", c=2)
        g3v = g3.rearrange("p t (r c) -> p t r c", c=2)
        tsc1 = scrp.tile([128, T, 3], f32, name="tsc1")
        tsc2 = scrp.tile([128, T, 3], f32, name="tsc2")

        ENTRIES = [r * 4 + c for r in range(3) for c in range(3)]

        def emit_rodrigues(th4, j0, nj):
            """theta block [j0, j0+nj) -> local rotation planes P['rXY'][:, :, :nj]."""
            s = lambda pl: P[pl][:, :, 0:nj]
            x = th4[:, :, j0 : j0 + nj, 0]
            y = th4[:, :, j0 : j0 + nj, 1]
            z = th4[:, :, j0 : j0 + nj, 2]
            if use_act:
                nc.scalar.activation(s("sqx"), x, Act.Square)
                nc.scalar.activation(s("sqy"), y, Act.Square)
                nc.scalar.activation(s("sqz"), z, Act.Square)
            else:
                nc.vector.tensor_mul(s("sqx"), x, x)
                nc.vector.tensor_mul(s("sqy"), y, y)
                nc.vector.tensor_mul(s("sqz"), z, z)
            nc.vector.tensor_add(s("nA"), s("sqx"), s("sqy"))
            nc.vector.tensor_add(s("n2"), s("nA"), s("sqz"))
            # powers of n2
            nc.vector.tensor_mul(s("u2"), s("n2"), s("n2"))
            if deg >= 3:
                nc.vector.tensor_mul(s("u3"), s("u2"), s("n2"))
            if deg >= 4:
                nc.vector.tensor_mul(s("u4"), s("u2"), s("u2"))
            if deg >= 5:
                nc.vector.tensor_mul(s("u5"), s("u3"), s("u2"))
            if deg >= 6:
                nc.vector.tensor_mul(s("u6"), s("u3"), s("u3"))
            if deg >= 7:
                nc.vector.tensor_mul(s("u7"), s("u4"), s("u3"))
            if deg >= 8:
                nc.vector.tensor_mul(s("u8"), s("u4"), s("u4"))

            def poly(coefs, a, b):
                # Evaluate sum coefs[i]*n2^i using ping-pong planes a, b.
                nc.vector.tensor_scalar(
                    s(a), s("n2"), float(coefs[1]), float(coefs[0]), Alu.mult, Alu.add
                )
                cur, nxt = a, b
                for i in range(2, len(coefs)):
                    nc.vector.scalar_tensor_tensor(
                        s(nxt), s("u%d" % i), float(coefs[i]), s(cur), Alu.mult, Alu.add
                    )
                    cur, nxt = nxt, cur
                return cur

            kf = poly(K_COEF, "kp", "kq")
            wf = poly(W_COEF, "wp", "wq")
            nc.vector.tensor_mul(s("qx"), x, s(kf))
            nc.vector.tensor_mul(s("qy"), y, s(kf))
            nc.vector.tensor_mul(s("qz"), z, s(kf))
            nc.vector.tensor_mul(s("pxy"), s("qx"), s("qy"))
            nc.vector.tensor_mul(s("pxz"), s("qx"), s("qz"))
            nc.vector.tensor_mul(s("pyz"), s("qy"), s("qz"))
            nc.vector.tensor_mul(s("wx"), s(wf), s("qx"))
            nc.vector.tensor_mul(s("wy"), s(wf), s("qy"))
            nc.vector.tensor_mul(s("wz"), s(wf), s("qz"))
            nc.vector.tensor_mul(s("pxx"), s("qx"), s("qx"))
            nc.vector.tensor_mul(s("pyy"), s("qy"), s("qy"))
            nc.vector.tensor_mul(s("pzz"), s("qz"), s("qz"))
            nc.vector.tensor_add(s("s1"), s("pyy"), s("pzz"))
            nc.vector.tensor_add(s("s2"), s("pxx"), s("pzz"))
            nc.vector.tensor_add(s("s3"), s("pxx"), s("pyy"))
            # R entry (r,c) lives at Rt[..., r*3+c]
            def rt(r, c):
                return Rt[:, :, 0:nj, r * 3 + c]

            # diagonals: r = 1 - 2*s
            if use_act:
                nc.scalar.activation(rt(0, 0), s("s1"), Act.Copy, bias=1.0, scale=-2.0)
                nc.scalar.activation(rt(1, 1), s("s2"), Act.Copy, bias=1.0, scale=-2.0)
                nc.scalar.activation(rt(2, 2), s("s3"), Act.Copy, bias=1.0, scale=-2.0)
            else:
                nc.vector.tensor_scalar(rt(0, 0), s("s1"), -2.0, 1.0, Alu.mult, Alu.add)
                nc.vector.tensor_scalar(rt(1, 1), s("s2"), -2.0, 1.0, Alu.mult, Alu.add)
                nc.vector.tensor_scalar(rt(2, 2), s("s3"), -2.0, 1.0, Alu.mult, Alu.add)
            # off-diagonals: wz/wy/wx are already 2*qw*q_
            nc.vector.scalar_tensor_tensor(
                rt(0, 1), s("pxy"), 2.0, s("wz"), Alu.mult, Alu.subtract
            )
            nc.vector.scalar_tensor_tensor(
                rt(1, 0), s("pxy"), 2.0, s("wz"), Alu.mult, Alu.add
            )
            nc.vector.scalar_tensor_tensor(
                rt(0, 2), s("pxz"), 2.0, s("wy"), Alu.mult, Alu.add
            )
            nc.vector.scalar_tensor_tensor(
                rt(2, 0), s("pxz"), 2.0, s("wy"), Alu.mult, Alu.subtract
            )
            nc.vector.scalar_tensor_tensor(
                rt(1, 2), s("pyz"), 2.0, s("wx"), Alu.mult, Alu.subtract
            )
            nc.vector.scalar_tensor_tensor(
                rt(2, 1), s("pyz"), 2.0, s("wx"), Alu.mult, Alu.add
            )

        def stage_of(j):
            return 0 if j < SEC_LOW[1] else 1

        def emit_group(c0, n, p0, pstep, blk_j0, stages5):
            """Compose children [c0, c0+n) from parents [p0, p0+n*pstep).

            stages5[k] is the 5D view [128, T, nj, 4, 4] of section k.
            """
            st_o, osec = stages5[stage_of(c0)], (SEC_LOW if c0 < 15 else SEC_HIGH)
            st_p, psec = stages5[stage_of(p0)], (SEC_LOW if p0 < 15 else SEC_HIGH)
            lo = c0 - blk_j0
            for i in range(n):
                j = c0 + i
                pj = p0 + i * pstep
                jo = j - osec[0]
                jp = pj - psec[0]

                # columns 0,1 of R_w: out[r,c] = sum_m par[r,m]*chl[m,c]
                # shapes [128, T, 3, 2]: r from parent (stride 4), c from child
                def parm(m):  # [128,T,3,1->2] parent column m over rows r
                    ap = st_p[:, :, jp, 0:3, m : m + 1]
                    return ap.broadcast_to((128, T, 3, 2))

                def chlm(m):  # [128,T,1->3,2] child row m, cols 0..1
                    ap = Rt[:, :, lo + i : lo + i + 1, m * 3 : m * 3 + 2]
                    return ap.broadcast_to((128, T, 3, 2))

                outc = st_o[:, :, jo, 0:3, 0:2]
                nc.vector.tensor_mul(g1v[:], parm(0), chlm(0))
                nc.vector.tensor_mul(g2v[:], parm(1), chlm(1))
                nc.vector.tensor_add(g3v[:], g1v[:], g2v[:])
                nc.vector.tensor_mul(g1v[:], parm(2), chlm(2))
                nc.vector.tensor_add(outc, g3v[:], g1v[:])
                # column 3: t_w = Rp @ bv + t_p (rows stacked, FD=3T)
                rp = lambda m: st_p[:, :, jp, 0:3, m]
                tp = st_p[:, :, jp, 0:3, 3]
                nc.vector.scalar_tensor_tensor(
                    tsc1[:], rp(0), cvcol(CV_BVX + j - 1), tp, Alu.mult, Alu.add
                )
                nc.vector.scalar_tensor_tensor(
                    tsc2[:], rp(1), cvcol(CV_BVY + j - 1), tsc1[:], Alu.mult, Alu.add
                )
                nc.vector.scalar_tensor_tensor(
                    st_o[:, :, jo, 0:3, 3],
                    rp(2),
                    cvcol(CV_BVZ + j - 1),
                    tsc2[:],
                    Alu.mult,
                    Alu.add,
                )
            # column 2 = col0 x col1 (reads back from stage), group-wide
            jo0 = c0 - osec[0]
            w = lambda r, c: st_o[:, :, jo0 : jo0 + n, r, c]
            for r in range(3):
                a1, a2 = (r + 1) % 3, (r + 2) % 3
                nc.vector.tensor_mul(g1[:, :, 0:n], w(a1, 0), w(a2, 1))
                nc.vector.tensor_mul(g2[:, :, 0:n], w(a2, 0), w(a1, 1))
                nc.vector.tensor_sub(w(r, 2), g1[:, :, 0:n], g2[:, :, 0:n])

        blk_of = {}
        for j0, nj in ROD_BLOCKS:
            for j in range(j0, j0 + nj):
                blk_of[j] = j0

        from contextlib import nullcontext

        # repeat>1: timing mode — run the whole pass `repeat` times via a
        # dynamic loop (same inputs/outputs each iteration).
        loop_cm = tc.For_i(0, repeat, 1) if repeat > 1 else nullcontext()
        with loop_cm:
            for ti in range(ntiles):
                th = thp.tile([128, T, 72], f32, name="th")
                nc.sync.dma_start(out=th[:], in_=theta_v[ti])
                oti = ti
                th4 = th.rearrange("p t (j c) -> p t j c", c=3)

                stageL = stp.tile([128, T, SL], f32, name="stageL")
                stageH = stp.tile([128, T, SH], f32, name="stageH")
                sL4 = stageL.rearrange("p t (j e) -> p t j e", e=16)
                sH4 = stageH.rearrange("p t (j e) -> p t j e", e=16)
                sL5 = stageL.rearrange("p t (j r c) -> p t j r c", r=4, c=4)
                sH5 = stageH.rearrange("p t (j r c) -> p t j r c", r=4, c=4)
                stages5 = [sL5, sH5]

                # constant bottom rows [0,0,0,1]
                mse = nc.gpsimd if use_gpsimd else nc.vector
                mse.memset(sL4[:, :, :, 12:15], 0.0)
                mse.memset(sL4[:, :, :, 15], 1.0)
                mse.memset(sH4[:, :, :, 12:15], 0.0)
                mse.memset(sH4[:, :, :, 15], 1.0)

                gi = 0
                for bi, (j0, nj) in enumerate(ROD_BLOCKS):
                    emit_rodrigues(th4, j0, nj)
                    if j0 == 0:
                        # root: l2w[0] = [R_loc[0] | t0]
                        for r in range(3):
                            nc.vector.tensor_copy(
                                sL5[:, :, 0, r, 0:3], Rt[:, :, 0, r * 3 : r * 3 + 3]
                            )
                        for r in range(3):
                            nc.vector.tensor_scalar(
                                sL4[:, :, 0, r * 4 + 3], P["n2"][:, :, 0],
                                0.0, cvcol(CV_T0 + r), Alu.mult, Alu.add,
                            )
                    # groups whose children live in this block
                    while gi < len(GROUPS) and blk_of[GROUPS[gi][0]] == j0:
                        c0, n, p0, pstep = GROUPS[gi]
                        emit_group(c0, n, p0, pstep, j0, stages5)
                        gi += 1
                    if (j0, nj) == (13, 2):
                        nc.sync.dma_start(
                            out=out_v[oti][:, :, 0:SL], in_=stageL[:]
                        )
                assert gi == len(GROUPS)
                nc.sync.dma_start(out=out_v[oti][:, :, SL : SL + SH], in_=stageH[:])

    nc.compile()
    _PROG_CACHE[key] = nc
    return nc


# --------------------------------------------------------------------------
# Host-side driver
# --------------------------------------------------------------------------


def _host_consts(rest_pose, bone_factor, parents):
    rp = np.asarray(rest_pose, dtype=np.float32)
    bf = np.asarray(bone_factor, dtype=np.float32).reshape(NJ - 1)
    par = np.asarray(parents, dtype=np.int64).reshape(NJ - 1)
    bfp = np.sqrt(bf * bf + np.float32(1e-36)).astype(np.float32)
    bv = (rp[1:] - rp[par]) * bfp[:, None]  # (23, 3)
    cval = np.zeros((128, CV_COLS), dtype=np.float32)
    cval[:, CV_BVX : CV_BVX + 23] = bv[:, 0]
    cval[:, CV_BVY : CV_BVY + 23] = bv[:, 1]
    cval[:, CV_BVZ : CV_BVZ + 23] = bv[:, 2]
    cval[:, CV_T0 : CV_T0 + 3] = rp[0]
    return cval


def kernel(theta, rest_pose, bone_factor, parents):
    from concourse.bass_utils import run_bass_kernel_spmd

    theta = np.ascontiguousarray(np.asarray(theta, dtype=np.float32))
    assert theta.shape == (B, NJ, 3)
    cval = _host_consts(rest_pose, bone_factor, parents)

    nc = build_program(T=64, ntiles=2, repeat=1)

    th_flat = theta.reshape(NCORES, BC, 72)
    in_maps = [{"theta_in": th_flat[c], "cval": cval} for c in range(NCORES)]
    res = run_bass_kernel_spmd(nc, in_maps, list(range(NCORES)))
    outs = [res.results[c]["out"] for c in range(NCORES)]
    l2ws = np.concatenate(outs, axis=0).reshape(B, NJ, 4, 4)
    kp3d = l2ws[..., :3, 3]
    orient = l2ws[..., :3, :3]
    return kp3d, orient, l2ws


# --------------------------------------------------------------------------
# Pure-numpy emulation of the device math (for simulator tests)
# --------------------------------------------------------------------------


def numpy_equivalent(theta, rest_pose, bone_factor, parents):
    """Same math as the device kernel, in float64-ish numpy (for debugging)."""
    th = np.asarray(theta, np.float32).astype(np.float64)
    Bd = th.shape[0]
    n2 = (th * th).sum(-1)  # (B, NJ)
    k = np.zeros_like(n2)
    w = np.zeros_like(n2)
    for i, c in enumerate(K_COEF):
        k += c * n2**i
    for i, c in enumerate(W_COEF):
        w += c * n2**i
    q = th * k[..., None]  # (B,NJ,3)
    qx, qy, qz = q[..., 0], q[..., 1], q[..., 2]
    wx, wy, wz = w * qx, w * qy, w * qz
    R = np.zeros((Bd, NJ, 3, 3))
    R[..., 0, 0] = 1 - 2 * (qy * qy + qz * qz)
    R[..., 1, 1] = 1 - 2 * (qx * qx + qz * qz)
    R[..., 2, 2] = 1 - 2 * (qx * qx + qy * qy)
    R[..., 0, 1] = 2 * qx * qy - wz
    R[..., 1, 0] = 2 * qx * qy + wz
    R[..., 0, 2] = 2 * qx * qz + wy
    R[..., 2, 0] = 2 * qx * qz - wy
    R[..., 1, 2] = 2 * qy * qz - wx
    R[..., 2, 1] = 2 * qy * qz + wx

    rp = np.asarray(rest_pose, np.float64)
    bf = np.asarray(bone_factor, np.float64).reshape(NJ - 1)
    par = np.asarray(parents, np.int64).reshape(NJ - 1)
    bv = (rp[1:] - rp[par]) * np.sqrt(bf * bf + 1e-36)[:, None]

    l2ws = np.zeros((Bd, NJ, 4, 4))
    l2ws[:, :, 3, 3] = 1.0
    l2ws[:, 0, :3, :3] = R[:, 0]
    l2ws[:, 0, :3, 3] = rp[0]
    for c0, n, p0, pstep in GROUPS:
        for i in range(n):
            j, pj = c0 + i, p0 + i * pstep
            Rw = l2ws[:, pj, :3, :3] @ R[:, j]
            Rw[:, :, 2] = np.cross(Rw[:, :, 0], Rw[:, :, 1])
            l2ws[:, j, :3, :3] = Rw
            l2ws[:, j, :3, 3] = (
                np.einsum("bij,j->bi", l2ws[:, pj, :3, :3], bv[j - 1])
                + l2ws[:, pj, :3, 3]
            )
    kp3d = l2ws[..., :3, 3]
    orient = l2ws[..., :3, :3]
    return kp3d.astype(np.float32), orient.astype(np.float32), l2ws.astype(np.float32)


if __name__ == "__main__":
    print(f"poly degree {POLY_DEG}, fit rel err {POLY_ERR:.2e}")
